# revision 8
# baseline (speedup 1.0000x reference)
"""Mistral decoder layer (S=2048, H=4096, NH=32, HD=128, FF=14336) on 8 TRN2
NeuronCores, tensor-parallel over heads / FF with feature-major ("transposed")
on-device layouts.

Per-core plan (core i of 8):
  - norm1 stats from the core's own 512-feature shard of hidden -> tiny AllReduce
  - x = rmsnorm(hidden) computed feature-major [H, S] (bf16)
  - q,k (feature-major [512, S]) and v (token-major [S, 512]) projections for
    the core's 4 heads; q pre-scaled by 1/sqrt(HD)
  - causal attention with unnormalized exp (scores are small; verified), key
    sums via ones-matmul, per-head normalization
  - o-proj partial [H, S] -> bf16 ReduceScatter over feature rows
  - h1 shard + norm2 (stats AllReduce) -> y shard -> AllGather -> full y [H, S]
  - MLP (gate/up/down on the core's 1792 FF rows) -> partial [H, S] -> bf16
    ReduceScatter -> + h1 shard -> output shard [512, S] fp32
Host assembles the 8 output shards and transposes back to [S, H].
"""

import sys
import types

sys.path.insert(0, "/opt/trn_rl_repo")

# Shim antenv.axon_hooks (absent in this container) so trace=True works.
import antenv  # noqa: E402

if "antenv.axon_hooks" not in sys.modules:
    _hooks_mod = types.ModuleType("antenv.axon_hooks")
    _hook_holder = [None]
    _hooks_mod.set_axon_ntff_profile_hook = lambda h: _hook_holder.__setitem__(0, h)
    _hooks_mod.get_axon_ntff_profile_hook = lambda: _hook_holder[0]
    sys.modules["antenv.axon_hooks"] = _hooks_mod
    antenv.axon_hooks = _hooks_mod
    try:
        from trn_agent_boot.trn_boot import _ntff_profile_via_ctypes

        _hooks_mod.set_axon_ntff_profile_hook(
            _ntff_profile_via_ctypes("/opt/axon/libaxon_pjrt.so")
        )
    except Exception:
        pass

import numpy as np  # noqa: E402
import ml_dtypes  # noqa: E402

import concourse.bass as bass  # noqa: E402
import concourse.mybir as mybir  # noqa: E402
import concourse.tile as tile  # noqa: E402
from concourse import bacc  # noqa: E402
from concourse.bass_utils import run_bass_kernel_spmd  # noqa: E402

BF16 = mybir.dt.bfloat16
F32 = mybir.dt.float32
AF = mybir.ActivationFunctionType
ALU = mybir.AluOpType
bfloat16 = ml_dtypes.bfloat16

S = 2048
H = 4096
NH = 32
HD = 128
FF = 14336
EPS = 1e-6
NC = 8
QK = H // NC          # 512: local q/k/v feature dim (4 heads)
LH = NH // NC         # 4 local heads
FFL = FF // NC        # 1792 local FF dim
SHD = H // NC         # 512: feature shard for RS/AG
KO = H // 128         # 32 contraction tiles over H
NT = S // 512         # 4 token chunks of 512
TCH = S // 128        # 16 token chunks of 128
FFC = FFL // 128      # 14
RG = [list(range(NC))]

_cache = {}


def _build(debug=False):
    nc = bacc.Bacc(None, target_bir_lowering=False, debug=False, num_devices=NC)

    # ---- inputs (per core) ----
    hsh = nc.dram_tensor("hsh", [128, LH, S], F32, kind="ExternalInput")   # own 512-feat shard of hiddenT
    hT = nc.dram_tensor("hT", [128, KO, S], F32, kind="ExternalInput")     # full hiddenT
    ln1w = nc.dram_tensor("ln1w", [128, KO, 1], F32, kind="ExternalInput")
    ln2w = nc.dram_tensor("ln2w", [128, LH, 1], F32, kind="ExternalInput")  # own shard
    wq = nc.dram_tensor("wq", [128, KO, QK], BF16, kind="ExternalInput")
    wk = nc.dram_tensor("wk", [128, KO, QK], BF16, kind="ExternalInput")
    wv = nc.dram_tensor("wv", [128, KO, QK], BF16, kind="ExternalInput")
    bq = nc.dram_tensor("bq", [128, LH, 1], F32, kind="ExternalInput")
    bk = nc.dram_tensor("bk", [128, LH, 1], F32, kind="ExternalInput")
    bvr = nc.dram_tensor("bvr", [1, QK], BF16, kind="ExternalInput")
    wo = nc.dram_tensor("wo", [128, LH, H], BF16, kind="ExternalInput")
    bo = nc.dram_tensor("bo", [128, LH, 1], F32, kind="ExternalInput")     # own shard
    wg = nc.dram_tensor("wg", [128, KO, FFL], BF16, kind="ExternalInput")
    wu = nc.dram_tensor("wu", [128, KO, FFL], BF16, kind="ExternalInput")
    wd = nc.dram_tensor("wd", [128, FFC, H], BF16, kind="ExternalInput")
    masks = nc.dram_tensor("masks", [128, 4, 512], BF16, kind="ExternalInput")

    out_sh = nc.dram_tensor("out_sh", [SHD, S], F32, kind="ExternalOutput")
    dbg = {}
    if debug:
        for name, shape, dt in [
            ("xT_dbg", [128, KO, S], BF16),
            ("q_dbg", [128, LH, S], BF16),
            ("k_dbg", [128, LH, S], BF16),
            ("v_dbg", [128, TCH, QK], BF16),
            ("hT_dbg", [128, LH, S], BF16),
            ("ors_dbg", [SHD, S], BF16),
            ("y_dbg", [H, S], BF16),
            ("mrs_dbg", [SHD, S], BF16),
        ]:
            dbg[name] = nc.dram_tensor(name, shape, dt, kind="ExternalOutput")

    with tile.TileContext(nc) as tc:
        with tc.tile_pool(name="dram", bufs=1, space="DRAM") as dram, \
             tc.tile_pool(name="pers", bufs=1) as sb:

            xT = dram.tile([128, KO, S], BF16, tag="xT")
            s1_in = dram.tile([1, S], F32, tag="s1i")
            s1_out = dram.tile([1, S], F32, tag="s1o", addr_space="Shared")
            o_cc_in = dram.tile([H, S], BF16, tag="occi")
            o_cc_out = dram.tile([SHD, S], BF16, tag="occo")
            s2_in = dram.tile([1, S], F32, tag="s2i")
            s2_out = dram.tile([1, S], F32, tag="s2o", addr_space="Shared")
            y_cc_in = dram.tile([SHD, S], BF16, tag="ycci")
            y_cc_out = dram.tile([H, S], BF16, tag="ycco", addr_space="Shared")
            d_cc_in = dram.tile([H, S], BF16, tag="dcci")
            d_cc_out = dram.tile([SHD, S], BF16, tag="dcco")

            # ---- persistent constants / long-lived tiles ----
            ones_col = sb.tile([1, 128], BF16, tag="ones_col")
            ones_red = sb.tile([128, 1], BF16, tag="ones_red")
            nc.vector.memset(ones_col[:], 1.0)
            nc.vector.memset(ones_red[:], 1.0)
            eps_t = sb.tile([1, 1], F32, tag="eps")
            nc.vector.memset(eps_t[:], EPS)
            mask_t = sb.tile([128, 4, 512], BF16, tag="mask")
            nc.sync.dma_start(mask_t[:], masks[:])
            bvr_t = sb.tile([1, QK], BF16, tag="bvr")
            nc.sync.dma_start(bvr_t[:], bvr[:])
            bq_t = sb.tile([128, LH, 1], F32, tag="bq")
            bk_t = sb.tile([128, LH, 1], F32, tag="bk")
            bo_t = sb.tile([128, LH, 1], F32, tag="bo")
            ln1_t = sb.tile([128, KO, 1], F32, tag="ln1")
            ln2_t = sb.tile([128, LH, 1], F32, tag="ln2")
            nc.sync.dma_start(bq_t[:], bq[:])
            nc.sync.dma_start(bk_t[:], bk[:])
            nc.sync.dma_start(bo_t[:], bo[:])
            nc.sync.dma_start(ln1_t[:], ln1w[:])
            nc.sync.dma_start(ln2_t[:], ln2w[:])

            h1_t = []  # 4x [128, S] f32, phases 7-11
            for j in range(LH):
                h1_t.append(sb.tile([128, S], F32, tag="h1", bufs=LH,
                                    name=f"h1_{j}"))

            # ================= phase 1+2: norm1 + x =================
            with tc.tile_pool(name="p12", bufs=1) as p12, \
                 tc.tile_pool(name="ps12", bufs=1, space="PSUM") as ps12:
                z1 = [ps12.tile([1, 512], F32, tag="z1", bufs=4, name=f"z1_{c}")
                      for c in range(4)]
                for j in range(LH):
                    hs = p12.tile([128, S], F32, tag="hshs", bufs=2)
                    nc.sync.dma_start(hs[:], hsh[:, j, :])
                    sq = p12.tile([128, S], BF16, tag="sq", bufs=2)
                    if j % 2 == 0:
                        nc.vector.tensor_tensor(sq[:], hs[:], hs[:], op=ALU.mult)
                    else:
                        nc.scalar.activation(sq[:], hs[:], AF.Square)
                    for c in range(4):
                        nc.tensor.matmul(z1[c][:], ones_red[:],
                                         sq[:, c * 512:(c + 1) * 512],
                                         start=(j == 0), stop=(j == LH - 1))
                s1row = p12.tile([1, S], F32, tag="row", bufs=3)
                for c in range(4):
                    nc.vector.tensor_copy(s1row[:, c * 512:(c + 1) * 512], z1[c][:])
                nc.sync.dma_start(s1_in[:], s1row[:])
                nc.gpsimd.collective_compute("AllReduce", ALU.add, replica_groups=RG,
                                             ins=[s1_in.opt()], outs=[s1_out.opt()])
                s1full = p12.tile([1, S], F32, tag="row", bufs=3)
                nc.sync.dma_start(s1full[:], s1_out[:])
                rms1 = p12.tile([1, S], F32, tag="row", bufs=3)
                nc.scalar.activation(rms1[:], s1full[:], AF.Sqrt, scale=1.0 / H,
                                     bias=eps_t[:])
                scl1 = p12.tile([1, S], F32, tag="row", bufs=3)
                nc.vector.reciprocal(scl1[:], rms1[:])
                sc1b = p12.tile([128, S], F32, tag="sc1b")
                nc.gpsimd.partition_broadcast(sc1b[:], scl1[:])

                for ko in range(KO):
                    ht = p12.tile([128, S], F32, tag="hfull", bufs=3)
                    nc.sync.dma_start(ht[:], hT[:, ko, :])
                    xt = p12.tile([128, S], BF16, tag="xw", bufs=3)
                    nc.vector.scalar_tensor_tensor(xt[:], ht[:], ln1_t[:, ko, :],
                                                   sc1b[:], op0=ALU.mult, op1=ALU.mult)
                    nc.sync.dma_start(xT[:, ko, :], xt[:])
                if debug:
                    nc.sync.dma_start(dbg["xT_dbg"][:], xT[:])

            # ============ phase 3+4+5: qkv, attention, o-proj ============
            with tc.tile_pool(name="p345", bufs=1) as p345:
                q_sl = p345.tile([128, LH, S], BF16, tag="q_sl")
                k_sl = p345.tile([128, LH, S], BF16, tag="k_sl")
                v_sl = p345.tile([128, TCH, QK], BF16, tag="v_sl")
                with tc.tile_pool(name="ps3", bufs=1, space="PSUM") as ps3:
                    for ntc in range(NT):
                        tsl = slice(ntc * 512, (ntc + 1) * 512)
                        xk = []
                        for ko in range(KO):
                            t = p345.tile([128, 512], BF16, tag="xk", bufs=KO + 2)
                            nc.sync.dma_start(t[:], xT[:, ko, tsl])
                            xk.append(t)
                        for (wdr, bias_t, dst) in ((wq, bq_t, q_sl), (wk, bk_t, k_sl)):
                            pq = [ps3.tile([128, 512], F32, tag="qkv", bufs=8,
                                           name=f"pq_{ntc}_{mc}") for mc in range(LH)]
                            for ko in range(KO):
                                wt = p345.tile([128, 512], BF16, tag="wqkv", bufs=4)
                                nc.sync.dma_start(wt[:], wdr[:, ko, :])
                                for mc in range(LH):
                                    nc.tensor.matmul(pq[mc][:],
                                                     wt[:, mc * 128:(mc + 1) * 128],
                                                     xk[ko][:], start=(ko == 0),
                                                     stop=(ko == KO - 1))
                            for mc in range(LH):
                                nc.scalar.activation(dst[:, mc, tsl], pq[mc][:],
                                                     AF.Identity, bias=bias_t[:, mc, :])
                        pv = [ps3.tile([128, 512], F32, tag="qkv", bufs=8,
                                       name=f"pv_{ntc}_{j}") for j in range(4)]
                        for j in range(4):
                            nc.tensor.matmul(pv[j][:], ones_col[:], bvr_t[:],
                                             start=True, stop=False)
                        for ko in range(KO):
                            wt = p345.tile([128, 512], BF16, tag="wqkv", bufs=4)
                            nc.sync.dma_start(wt[:], wv[:, ko, :])
                            for j in range(4):
                                nc.tensor.matmul(pv[j][:],
                                                 xk[ko][:, j * 128:(j + 1) * 128],
                                                 wt[:], start=False,
                                                 stop=(ko == KO - 1))
                        for j in range(4):
                            nc.vector.tensor_copy(v_sl[:, ntc * 4 + j, :], pv[j][:])
                if debug:
                    nc.sync.dma_start(dbg["q_dbg"][:], q_sl[:])
                    nc.sync.dma_start(dbg["k_dbg"][:], k_sl[:])
                    nc.sync.dma_start(dbg["v_dbg"][:], v_sl[:])

                hT_sl = p345.tile([128, LH, S], BF16, tag="hT_sl")
                with tc.tile_pool(name="ps4", bufs=1, space="PSUM") as ps4:
                    for h in range(LH):
                        for qc in range(NT):
                            qsl = slice(qc * 512, (qc + 1) * 512)
                            kc_max = 4 * qc + 3
                            pz = ps4.tile([1, 512], F32, tag="pz", bufs=2)
                            ph = ps4.tile([128, 512], F32, tag="ph", bufs=2)
                            for kc in range(kc_max + 1):
                                pscr = ps4.tile([128, 512], F32, tag="pscr", bufs=2)
                                nc.tensor.matmul(pscr[:],
                                                 k_sl[:, h, kc * 128:(kc + 1) * 128],
                                                 q_sl[:, h, qsl], start=True, stop=True)
                                probs = p345.tile([128, 512], BF16, tag="probs", bufs=4)
                                nc.scalar.activation(probs[:], pscr[:], AF.Exp)
                                if kc >= 4 * qc:
                                    nc.vector.tensor_tensor(probs[:], probs[:],
                                                            mask_t[:, kc - 4 * qc, :],
                                                            op=ALU.mult)
                                nc.tensor.matmul(pz[:], ones_red[:], probs[:],
                                                 start=(kc == 0), stop=(kc == kc_max))
                                nc.tensor.matmul(ph[:],
                                                 v_sl[:, kc, h * 128:(h + 1) * 128],
                                                 probs[:], start=(kc == 0),
                                                 stop=(kc == kc_max))
                            rz = p345.tile([1, 512], F32, tag="rz", bufs=2)
                            nc.vector.reciprocal(rz[:], pz[:])
                            rzb = p345.tile([128, 512], F32, tag="rzb", bufs=2)
                            nc.gpsimd.partition_broadcast(rzb[:], rz[:])
                            nc.vector.tensor_tensor(hT_sl[:, h, qsl], ph[:], rzb[:],
                                                    op=ALU.mult)
                if debug:
                    nc.sync.dma_start(dbg["hT_dbg"][:], hT_sl[:])

                with tc.tile_pool(name="ps5", bufs=1, space="PSUM") as ps5:
                    for ntc in range(NT):
                        tsl = slice(ntc * 512, (ntc + 1) * 512)
                        for mc in range(KO):
                            wot = p345.tile([128, LH, 128], BF16, tag="wot", bufs=4)
                            nc.sync.dma_start(wot[:], wo[:, :, mc * 128:(mc + 1) * 128])
                            po = ps5.tile([128, 512], F32, tag="po", bufs=3)
                            for ko in range(LH):
                                nc.tensor.matmul(po[:], wot[:, ko, :],
                                                 hT_sl[:, ko, tsl],
                                                 start=(ko == 0), stop=(ko == LH - 1))
                            oo = p345.tile([128, 512], BF16, tag="oo", bufs=4)
                            nc.vector.tensor_copy(oo[:], po[:])
                            nc.sync.dma_start(o_cc_in[mc * 128:(mc + 1) * 128, tsl],
                                              oo[:])

            # ================= phase 6: ReduceScatter o =================
            nc.gpsimd.collective_compute("ReduceScatter", ALU.add, replica_groups=RG,
                                         ins=[o_cc_in.opt()], outs=[o_cc_out.opt()])
            if debug:
                nc.sync.dma_start(dbg["ors_dbg"][:], o_cc_out[:])

            # ========= phase 7+8: h1, norm2, y shard, AllGather =========
            with tc.tile_pool(name="p78", bufs=1) as p78, \
                 tc.tile_pool(name="ps78", bufs=1, space="PSUM") as ps78:
                z2 = [ps78.tile([1, 512], F32, tag="z2", bufs=4, name=f"z2_{c}")
                      for c in range(4)]
                for j in range(LH):
                    osh = p78.tile([128, S], BF16, tag="osh", bufs=2)
                    nc.sync.dma_start(osh[:], o_cc_out[j * 128:(j + 1) * 128, :])
                    hs = p78.tile([128, S], F32, tag="hshs", bufs=2)
                    nc.sync.dma_start(hs[:], hsh[:, j, :])
                    nc.vector.scalar_tensor_tensor(h1_t[j][:], osh[:], bo_t[:, j, :],
                                                   hs[:], op0=ALU.add, op1=ALU.add)
                    sq = p78.tile([128, S], BF16, tag="sq", bufs=2)
                    nc.scalar.activation(sq[:], h1_t[j][:], AF.Square)
                    for c in range(4):
                        nc.tensor.matmul(z2[c][:], ones_red[:],
                                         sq[:, c * 512:(c + 1) * 512],
                                         start=(j == 0), stop=(j == LH - 1))
                s2row = p78.tile([1, S], F32, tag="row", bufs=3)
                for c in range(4):
                    nc.vector.tensor_copy(s2row[:, c * 512:(c + 1) * 512], z2[c][:])
                nc.sync.dma_start(s2_in[:], s2row[:])
                nc.gpsimd.collective_compute("AllReduce", ALU.add, replica_groups=RG,
                                             ins=[s2_in.opt()], outs=[s2_out.opt()])
                s2full = p78.tile([1, S], F32, tag="row", bufs=3)
                nc.sync.dma_start(s2full[:], s2_out[:])
                rms2 = p78.tile([1, S], F32, tag="row", bufs=3)
                nc.scalar.activation(rms2[:], s2full[:], AF.Sqrt, scale=1.0 / H,
                                     bias=eps_t[:])
                scl2 = p78.tile([1, S], F32, tag="row", bufs=3)
                nc.vector.reciprocal(scl2[:], rms2[:])
                sc2b = p78.tile([128, S], F32, tag="sc2b")
                nc.gpsimd.partition_broadcast(sc2b[:], scl2[:])
                for j in range(LH):
                    ysh = p78.tile([128, S], BF16, tag="ysh", bufs=2)
                    nc.vector.scalar_tensor_tensor(ysh[:], h1_t[j][:], ln2_t[:, j, :],
                                                   sc2b[:], op0=ALU.mult, op1=ALU.mult)
                    nc.sync.dma_start(y_cc_in[j * 128:(j + 1) * 128, :], ysh[:])
            nc.gpsimd.collective_compute("AllGather", ALU.bypass, replica_groups=RG,
                                         ins=[y_cc_in.opt()], outs=[y_cc_out.opt()])
            if debug:
                nc.sync.dma_start(dbg["y_dbg"][:], y_cc_out[:])

            # ================= phase 9+10+11: MLP, RS, out =================
            with tc.tile_pool(name="p9", bufs=1) as p9, \
                 tc.tile_pool(name="ps9", bufs=1, space="PSUM") as ps9:
                for ntc in range(NT):
                    tsl = slice(ntc * 512, (ntc + 1) * 512)
                    yk = []
                    for ko in range(KO):
                        t = p9.tile([128, 512], BF16, tag="yk", bufs=KO + 2)
                        nc.sync.dma_start(t[:], y_cc_out[ko * 128:(ko + 1) * 128, tsl])
                        yk.append(t)
                    act_sl = p9.tile([128, FFC, 512], BF16, tag="act", bufs=2)
                    for fc in range(FFC):
                        wgt = p9.tile([128, KO, 128], BF16, tag="wgu", bufs=3)
                        nc.sync.dma_start(wgt[:], wg[:, :, fc * 128:(fc + 1) * 128])
                        wut = p9.tile([128, KO, 128], BF16, tag="wgu", bufs=3)
                        nc.sync.dma_start(wut[:], wu[:, :, fc * 128:(fc + 1) * 128])
                        pg = ps9.tile([128, 512], F32, tag="pg", bufs=2)
                        pu = ps9.tile([128, 512], F32, tag="pu", bufs=2)
                        for ko in range(KO):
                            nc.tensor.matmul(pg[:], wgt[:, ko, :], yk[ko][:],
                                             start=(ko == 0), stop=(ko == KO - 1))
                        for ko in range(KO):
                            nc.tensor.matmul(pu[:], wut[:, ko, :], yk[ko][:],
                                             start=(ko == 0), stop=(ko == KO - 1))
                        sg = p9.tile([128, 512], F32, tag="sg", bufs=2)
                        nc.scalar.activation(sg[:], pg[:], AF.Silu)
                        nc.vector.tensor_tensor(act_sl[:, fc, :], pu[:], sg[:],
                                                op=ALU.mult)
                    for mc in range(KO):
                        wdt = p9.tile([128, FFC, 128], BF16, tag="wdt", bufs=4)
                        nc.sync.dma_start(wdt[:], wd[:, :, mc * 128:(mc + 1) * 128])
                        pd = ps9.tile([128, 512], F32, tag="pd", bufs=3)
                        for fc in range(FFC):
                            nc.tensor.matmul(pd[:], wdt[:, fc, :], act_sl[:, fc, :],
                                             start=(fc == 0), stop=(fc == FFC - 1))
                        dd = p9.tile([128, 512], BF16, tag="dd", bufs=4)
                        nc.scalar.copy(dd[:], pd[:])
                        nc.sync.dma_start(d_cc_in[mc * 128:(mc + 1) * 128, tsl], dd[:])

                nc.gpsimd.collective_compute("ReduceScatter", ALU.add,
                                             replica_groups=RG,
                                             ins=[d_cc_in.opt()], outs=[d_cc_out.opt()])
                if debug:
                    nc.sync.dma_start(dbg["mrs_dbg"][:], d_cc_out[:])

                for j in range(LH):
                    msh = p9.tile([128, S], BF16, tag="msh", bufs=2)
                    nc.sync.dma_start(msh[:], d_cc_out[j * 128:(j + 1) * 128, :])
                    ot = p9.tile([128, S], F32, tag="outt", bufs=2)
                    nc.vector.tensor_tensor(ot[:], h1_t[j][:], msh[:], op=ALU.add)
                    nc.sync.dma_start(out_sh[j * 128:(j + 1) * 128, :], ot[:])

    nc.compile()
    return nc


def _feat_major(a):
    """[Hin, M] fp32 -> [128, Hin//128, M]"""
    hin, m = a.shape
    return np.ascontiguousarray(a.reshape(hin // 128, 128, m).swapaxes(0, 1))


def _col(b):
    """[512] -> [128, 4, 1]"""
    return np.ascontiguousarray(b.reshape(-1, 128, 1).swapaxes(0, 1))


def _prep_inputs(hidden_states, wq, bq, wk, bk, wv, bv, wo, bo,
                 w_gate, w_up, w_down, ln1_w, ln2_w):
    f32 = np.float32
    hidden = np.asarray(hidden_states, f32)
    hTn = _feat_major(np.ascontiguousarray(hidden.T))           # [128, 32, S]
    ln1 = np.asarray(ln1_w, f32).reshape(KO, 128, 1).swapaxes(0, 1).copy()
    scale = 1.0 / np.sqrt(HD)

    mask = np.zeros((128, 4, 512), f32)
    p = np.arange(128)[:, None, None]
    j = np.arange(4)[None, :, None]
    c = np.arange(512)[None, None, :]
    mask[c >= p + 128 * j] = 1.0
    mask = mask.astype(bfloat16)

    wq_ = np.asarray(wq, f32) * scale
    bq_ = np.asarray(bq, f32) * scale
    wk_, bk_ = np.asarray(wk, f32), np.asarray(bk, f32)
    wv_, bv_ = np.asarray(wv, f32), np.asarray(bv, f32)
    wo_, bo_ = np.asarray(wo, f32), np.asarray(bo, f32)
    wg_, wu_, wdn_ = (np.asarray(w_gate, f32), np.asarray(w_up, f32),
                      np.asarray(w_down, f32))
    ln2 = np.asarray(ln2_w, f32)

    in_maps = []
    for i in range(NC):
        qs = slice(i * QK, (i + 1) * QK)
        fs = slice(i * FFL, (i + 1) * FFL)
        ss = slice(i * SHD, (i + 1) * SHD)
        m = {
            "hsh": _feat_major(np.ascontiguousarray(hidden.T[ss, :])),
            "hT": hTn,
            "ln1w": ln1,
            "ln2w": _col(ln2[ss]),
            "wq": _feat_major(wq_[qs, :].T).astype(bfloat16),
            "wk": _feat_major(wk_[qs, :].T).astype(bfloat16),
            "wv": _feat_major(wv_[qs, :].T).astype(bfloat16),
            "bq": _col(bq_[qs]),
            "bk": _col(bk_[qs]),
            "bvr": bv_[qs][None, :].astype(bfloat16),
            "wo": _feat_major(wo_[:, qs].T).astype(bfloat16),
            "bo": _col(bo_[ss]),
            "wg": _feat_major(wg_[fs, :].T).astype(bfloat16),
            "wu": _feat_major(wu_[fs, :].T).astype(bfloat16),
            "wd": _feat_major(wdn_[:, fs].T).astype(bfloat16),
            "masks": mask,
        }
        in_maps.append(m)
    return in_maps


def run(inputs, debug=False, trace=False):
    key = ("nc", debug)
    if key not in _cache:
        _cache[key] = _build(debug=debug)
    nc = _cache[key]
    in_maps = _prep_inputs(
        inputs["hidden_states"], inputs["wq"], inputs["bq"], inputs["wk"],
        inputs["bk"], inputs["wv"], inputs["bv"], inputs["wo"], inputs["bo"],
        inputs["w_gate"], inputs["w_up"], inputs["w_down"], inputs["ln1_w"],
        inputs["ln2_w"])
    res = run_bass_kernel_spmd(nc, in_maps, core_ids=list(range(NC)), trace=trace)
    shards = [np.asarray(r["out_sh"]) for r in res.results]
    out = np.concatenate(shards, axis=0).T
    return np.ascontiguousarray(out, dtype=np.float32), res


def kernel(**inputs):
    out, _ = run(inputs, debug=False, trace=False)
    return out


# revision 10
# speedup vs baseline: 1.1390x; 1.1390x over previous
"""Mistral decoder layer (S=2048, H=4096, NH=32, HD=128, FF=14336) on 8 TRN2
NeuronCores, tensor-parallel over heads / FF with feature-major ("transposed")
on-device layouts.

Per-core plan (core i of 8):
  - norm1 stats from the core's own 512-feature shard of hidden -> tiny AllReduce
  - x = rmsnorm(hidden) computed feature-major on the fly (bf16), no DRAM trip
  - q,k (feature-major [512, S]) and v (token-major [S, 512]) projections for
    the core's 4 heads; q pre-scaled by 1/sqrt(HD)
  - causal attention with unnormalized exp (scores are small; verified), key
    sums via ones-matmul, per-head normalization
  - per 512-token chunk: o-proj partial [H, 512] -> bf16 ReduceScatter over
    feature rows (overlaps attention of later chunks)
  - per chunk: h1 shard + norm2 (stats AllReduce) -> y shard -> AllGather
  - per chunk: MLP (gate/up/down on the core's 1792 FF rows) -> partial
    [H, 512] -> bf16 ReduceScatter -> + h1 shard -> output shard fp32
Host assembles the 8 output shards and transposes back to [S, H].
"""

import sys
import types

sys.path.insert(0, "/opt/trn_rl_repo")

# Shim antenv.axon_hooks (absent in this container) so trace=True works.
import antenv  # noqa: E402

if "antenv.axon_hooks" not in sys.modules:
    _hooks_mod = types.ModuleType("antenv.axon_hooks")
    _hook_holder = [None]
    _hooks_mod.set_axon_ntff_profile_hook = lambda h: _hook_holder.__setitem__(0, h)
    _hooks_mod.get_axon_ntff_profile_hook = lambda: _hook_holder[0]
    sys.modules["antenv.axon_hooks"] = _hooks_mod
    antenv.axon_hooks = _hooks_mod
    try:
        from trn_agent_boot.trn_boot import _ntff_profile_via_ctypes

        _hooks_mod.set_axon_ntff_profile_hook(
            _ntff_profile_via_ctypes("/opt/axon/libaxon_pjrt.so")
        )
    except Exception:
        pass

import numpy as np  # noqa: E402
import ml_dtypes  # noqa: E402

import concourse.bass as bass  # noqa: E402
import concourse.mybir as mybir  # noqa: E402
import concourse.tile as tile  # noqa: E402
from concourse import bacc  # noqa: E402
from concourse.bass_utils import run_bass_kernel_spmd  # noqa: E402

BF16 = mybir.dt.bfloat16
F32 = mybir.dt.float32
AF = mybir.ActivationFunctionType
ALU = mybir.AluOpType
bfloat16 = ml_dtypes.bfloat16

S = 2048
H = 4096
NH = 32
HD = 128
FF = 14336
EPS = 1e-6
NC = 8
QK = H // NC          # 512: local q/k/v feature dim (4 heads)
LH = NH // NC         # 4 local heads
FFL = FF // NC        # 1792 local FF dim
SHD = H // NC         # 512: feature shard for RS/AG
KO = H // 128         # 32 contraction tiles over H
NT = S // 512         # 4 token chunks of 512
TCH = S // 128        # 16 token chunks of 128
FFC = FFL // 128      # 14
RG = [list(range(NC))]

_cache = {}


def _build(debug=False):
    nc = bacc.Bacc(None, target_bir_lowering=False, debug=False, num_devices=NC)

    # ---- inputs (per core) ----
    hsh = nc.dram_tensor("hsh", [128, LH, S], F32, kind="ExternalInput")
    hT = nc.dram_tensor("hT", [128, KO, S], F32, kind="ExternalInput")
    ln1w = nc.dram_tensor("ln1w", [128, KO, 1], F32, kind="ExternalInput")
    ln2w = nc.dram_tensor("ln2w", [128, LH, 1], F32, kind="ExternalInput")
    wq = nc.dram_tensor("wq", [128, KO, QK], BF16, kind="ExternalInput")
    wk = nc.dram_tensor("wk", [128, KO, QK], BF16, kind="ExternalInput")
    wv = nc.dram_tensor("wv", [128, KO, QK], BF16, kind="ExternalInput")
    bq = nc.dram_tensor("bq", [128, LH, 1], F32, kind="ExternalInput")
    bk = nc.dram_tensor("bk", [128, LH, 1], F32, kind="ExternalInput")
    bvr = nc.dram_tensor("bvr", [1, QK], BF16, kind="ExternalInput")
    # wo: [p, mc(32), ko(4), 128] -> contiguous [128, 4, 128] per-mc slices
    wo = nc.dram_tensor("wo", [128, KO, LH, 128], BF16, kind="ExternalInput")
    bo = nc.dram_tensor("bo", [128, LH, 1], F32, kind="ExternalInput")
    # wg/wu: [p, fc(14), ko(32), 128]; wd: [p, mc(32), fc(14), 128]
    wg = nc.dram_tensor("wg", [128, FFC, KO, 128], BF16, kind="ExternalInput")
    wu = nc.dram_tensor("wu", [128, FFC, KO, 128], BF16, kind="ExternalInput")
    wd = nc.dram_tensor("wd", [128, KO, FFC, 128], BF16, kind="ExternalInput")
    masks = nc.dram_tensor("masks", [128, 4, 512], BF16, kind="ExternalInput")

    out_sh = nc.dram_tensor("out_sh", [SHD, S], F32, kind="ExternalOutput")
    dbg = {}
    if debug:
        for name, shape, dt in [
            ("q_dbg", [128, LH, S], BF16),
            ("k_dbg", [128, LH, S], BF16),
            ("v_dbg", [128, TCH, QK], BF16),
            ("hT_dbg", [128, LH, S], BF16),
            ("ors_dbg", [SHD, S], BF16),
            ("y_dbg", [H, S], BF16),
            ("mrs_dbg", [SHD, S], BF16),
        ]:
            dbg[name] = nc.dram_tensor(name, shape, dt, kind="ExternalOutput")

    with tile.TileContext(nc) as tc:
        with tc.tile_pool(name="dram", bufs=1, space="DRAM") as dram, \
             tc.tile_pool(name="pers", bufs=1) as sb, \
             tc.tile_pool(name="pp", bufs=1, space="PSUM") as pp:

            s1_in = dram.tile([1, S], F32, tag="s1i")
            s1_out = dram.tile([1, S], F32, tag="s1o", addr_space="Shared")
            o_in_c = [dram.tile([H, 512], BF16, tag="occi", bufs=NT,
                                name=f"o_in_{c}") for c in range(NT)]
            o_out_c = [dram.tile([SHD, 512], BF16, tag="occo", bufs=NT,
                                 name=f"o_out_{c}") for c in range(NT)]
            s2_in_c = [dram.tile([1, 512], F32, tag="s2i", bufs=NT,
                                 name=f"s2_in_{c}") for c in range(NT)]
            s2_out_c = [dram.tile([1, 512], F32, tag="s2o", bufs=NT,
                                  addr_space="Shared", name=f"s2_out_{c}")
                        for c in range(NT)]
            y_in_c = [dram.tile([SHD, 512], BF16, tag="ycci", bufs=NT,
                                name=f"y_in_{c}") for c in range(NT)]
            y_out_c = [dram.tile([H, 512], BF16, tag="ycco", bufs=NT,
                                 addr_space="Shared", name=f"y_out_{c}")
                       for c in range(NT)]
            d_in_c = [dram.tile([H, 512], BF16, tag="dcci", bufs=NT,
                                name=f"d_in_{c}") for c in range(NT)]
            d_out_c = [dram.tile([SHD, 512], BF16, tag="dcco", bufs=NT,
                                 name=f"d_out_{c}") for c in range(NT)]

            # ---- persistent constants / long-lived tiles ----
            ones_col = sb.tile([1, 128], BF16, tag="ones_col")
            ones_red = sb.tile([128, 1], BF16, tag="ones_red")
            nc.vector.memset(ones_col[:], 1.0)
            nc.vector.memset(ones_red[:], 1.0)
            eps_t = sb.tile([1, 1], F32, tag="eps")
            nc.vector.memset(eps_t[:], EPS)
            mask_t = sb.tile([128, 4, 512], BF16, tag="mask")
            nc.sync.dma_start(mask_t[:], masks[:])
            bvr_t = sb.tile([1, QK], BF16, tag="bvr")
            nc.sync.dma_start(bvr_t[:], bvr[:])
            bq_t = sb.tile([128, LH, 1], F32, tag="bq")
            bk_t = sb.tile([128, LH, 1], F32, tag="bk")
            bo_t = sb.tile([128, LH, 1], F32, tag="bo")
            ln1_t = sb.tile([128, KO, 1], F32, tag="ln1")
            ln2_t = sb.tile([128, LH, 1], F32, tag="ln2")
            nc.sync.dma_start(bq_t[:], bq[:])
            nc.sync.dma_start(bk_t[:], bk[:])
            nc.sync.dma_start(bo_t[:], bo[:])
            nc.sync.dma_start(ln1_t[:], ln1w[:])
            nc.sync.dma_start(ln2_t[:], ln2w[:])

            h1_t = [sb.tile([128, S], F32, tag="h1", bufs=LH, name=f"h1_{j}")
                    for j in range(LH)]
            sc1b = sb.tile([128, S], F32, tag="sc1b")

            # ================= norm1 stats + AllReduce =================
            with tc.tile_pool(name="p1", bufs=1) as p1:
                z1 = [pp.tile([1, 512], F32, tag="pp", bufs=8, name=f"z1_{c}")
                      for c in range(4)]
                for j in range(LH):
                    hs = p1.tile([128, S], F32, tag="hshs", bufs=2)
                    nc.sync.dma_start(hs[:], hsh[:, j, :])
                    sq = p1.tile([128, S], BF16, tag="sq", bufs=2)
                    if j % 2 == 0:
                        nc.vector.tensor_tensor(sq[:], hs[:], hs[:], op=ALU.mult)
                    else:
                        nc.scalar.activation(sq[:], hs[:], AF.Square)
                    for c in range(4):
                        nc.tensor.matmul(z1[c][:], ones_red[:],
                                         sq[:, c * 512:(c + 1) * 512],
                                         start=(j == 0), stop=(j == LH - 1))
                s1row = p1.tile([1, S], F32, tag="row", bufs=2)
                for c in range(4):
                    nc.vector.tensor_copy(s1row[:, c * 512:(c + 1) * 512], z1[c][:])
                nc.sync.dma_start(s1_in[:], s1row[:])
                nc.gpsimd.collective_compute("AllReduce", ALU.add, replica_groups=RG,
                                             ins=[s1_in.opt()], outs=[s1_out.opt()])
                s1full = p1.tile([1, S], F32, tag="row", bufs=2)
                nc.sync.dma_start(s1full[:], s1_out[:])
                rms1 = p1.tile([1, S], F32, tag="rms1")
                nc.scalar.activation(rms1[:], s1full[:], AF.Sqrt, scale=1.0 / H,
                                     bias=eps_t[:])
                scl1 = p1.tile([1, S], F32, tag="scl1")
                nc.vector.reciprocal(scl1[:], rms1[:])
                nc.gpsimd.partition_broadcast(sc1b[:], scl1[:])

            # ============ qkv + attention + o-proj + chunked RS ============
            with tc.tile_pool(name="p345", bufs=1) as p345:
                q_sl = p345.tile([128, LH, S], BF16, tag="q_sl")
                k_sl = p345.tile([128, LH, S], BF16, tag="k_sl")
                v_sl = p345.tile([128, TCH, QK], BF16, tag="v_sl")
                for ntc in range(NT):
                    tsl = slice(ntc * 512, (ntc + 1) * 512)
                    xk = []
                    for ko in range(KO):
                        hf = p345.tile([128, 512], F32, tag="hf", bufs=4)
                        nc.sync.dma_start(hf[:], hT[:, ko, tsl])
                        t = p345.tile([128, 512], BF16, tag="xk", bufs=KO + 2)
                        nc.vector.scalar_tensor_tensor(t[:], hf[:], ln1_t[:, ko, :],
                                                       sc1b[:, tsl],
                                                       op0=ALU.mult, op1=ALU.mult)
                        xk.append(t)
                    for (wdr, bias_t, dst) in ((wq, bq_t, q_sl), (wk, bk_t, k_sl)):
                        pq = [pp.tile([128, 512], F32, tag="pp", bufs=8,
                                      name=f"pq_{ntc}_{mc}_{dst.tensor.name}")
                              for mc in range(LH)]
                        for ko in range(KO):
                            wt = p345.tile([128, 512], BF16, tag="wqkv", bufs=6)
                            nc.sync.dma_start(wt[:], wdr[:, ko, :])
                            for mc in range(LH):
                                nc.tensor.matmul(pq[mc][:],
                                                 wt[:, mc * 128:(mc + 1) * 128],
                                                 xk[ko][:], start=(ko == 0),
                                                 stop=(ko == KO - 1))
                        for mc in range(LH):
                            nc.scalar.activation(dst[:, mc, tsl], pq[mc][:],
                                                 AF.Identity, bias=bias_t[:, mc, :])
                    pv = [pp.tile([128, 512], F32, tag="pp", bufs=8,
                                  name=f"pv_{ntc}_{j}") for j in range(4)]
                    for j in range(4):
                        nc.tensor.matmul(pv[j][:], ones_col[:], bvr_t[:],
                                         start=True, stop=False)
                    for ko in range(KO):
                        wt = p345.tile([128, 512], BF16, tag="wqkv", bufs=6)
                        nc.sync.dma_start(wt[:], wv[:, ko, :])
                        for j in range(4):
                            nc.tensor.matmul(pv[j][:],
                                             xk[ko][:, j * 128:(j + 1) * 128],
                                             wt[:], start=False,
                                             stop=(ko == KO - 1))
                    for j in range(4):
                        nc.vector.tensor_copy(v_sl[:, ntc * 4 + j, :], pv[j][:])
                if debug:
                    nc.sync.dma_start(dbg["q_dbg"][:], q_sl[:])
                    nc.sync.dma_start(dbg["k_dbg"][:], k_sl[:])
                    nc.sync.dma_start(dbg["v_dbg"][:], v_sl[:])

                hT_sl = p345.tile([128, LH, S], BF16, tag="hT_sl")
                for qc in range(NT):
                    qsl = slice(qc * 512, (qc + 1) * 512)
                    kc_max = 4 * qc + 3
                    for h in range(LH):
                        pz = pp.tile([1, 512], F32, tag="pp", bufs=8,
                                     name=f"pz_{qc}_{h}")
                        ph = pp.tile([128, 512], F32, tag="pp", bufs=8,
                                     name=f"ph_{qc}_{h}")
                        for kc in range(kc_max + 1):
                            pscr = pp.tile([128, 512], F32, tag="pp", bufs=8,
                                           name=f"ps_{qc}_{h}_{kc}")
                            nc.tensor.matmul(pscr[:],
                                             k_sl[:, h, kc * 128:(kc + 1) * 128],
                                             q_sl[:, h, qsl], start=True, stop=True)
                            probs = p345.tile([128, 512], BF16, tag="probs", bufs=6)
                            nc.scalar.activation(probs[:], pscr[:], AF.Exp)
                            if kc >= 4 * qc:
                                nc.vector.tensor_tensor(probs[:], probs[:],
                                                        mask_t[:, kc - 4 * qc, :],
                                                        op=ALU.mult)
                            nc.tensor.matmul(pz[:], ones_red[:], probs[:],
                                             start=(kc == 0), stop=(kc == kc_max))
                            nc.tensor.matmul(ph[:],
                                             v_sl[:, kc, h * 128:(h + 1) * 128],
                                             probs[:], start=(kc == 0),
                                             stop=(kc == kc_max))
                        rz = p345.tile([1, 512], F32, tag="rz", bufs=2)
                        nc.vector.reciprocal(rz[:], pz[:])
                        rzb = p345.tile([128, 512], F32, tag="rzb", bufs=2)
                        nc.gpsimd.partition_broadcast(rzb[:], rz[:])
                        nc.vector.tensor_tensor(hT_sl[:, h, qsl], ph[:], rzb[:],
                                                op=ALU.mult)
                    # o-proj for this token chunk, then RS it
                    for mc in range(KO):
                        wot = p345.tile([128, LH, 128], BF16, tag="wot", bufs=4)
                        nc.sync.dma_start(wot[:], wo[:, mc, :, :])
                        po = pp.tile([128, 512], F32, tag="pp", bufs=8,
                                     name=f"po_{qc}_{mc}")
                        for ko in range(LH):
                            nc.tensor.matmul(po[:], wot[:, ko, :], hT_sl[:, ko, qsl],
                                             start=(ko == 0), stop=(ko == LH - 1))
                        oo = p345.tile([128, 512], BF16, tag="oo", bufs=4)
                        nc.vector.tensor_copy(oo[:], po[:])
                        nc.sync.dma_start(o_in_c[qc][mc * 128:(mc + 1) * 128, :],
                                          oo[:])
                    nc.gpsimd.collective_compute(
                        "ReduceScatter", ALU.add, replica_groups=RG,
                        ins=[o_in_c[qc].opt()], outs=[o_out_c[qc].opt()])
                if debug:
                    nc.sync.dma_start(dbg["hT_dbg"][:], hT_sl[:])

                # ===== h1 + norm2 + y shard + AllGather, per chunk =====
                with tc.tile_pool(name="p78", bufs=1) as p78:
                    for qc in range(NT):
                        qsl = slice(qc * 512, (qc + 1) * 512)
                        if debug:
                            nc.sync.dma_start(dbg["ors_dbg"][:, qsl], o_out_c[qc][:])
                        z2 = pp.tile([1, 512], F32, tag="pp", bufs=8,
                                     name=f"z2_{qc}")
                        for j in range(LH):
                            osh = p78.tile([128, 512], BF16, tag="osh", bufs=2)
                            nc.sync.dma_start(osh[:],
                                              o_out_c[qc][j * 128:(j + 1) * 128, :])
                            hs = p78.tile([128, 512], F32, tag="hshc", bufs=2)
                            nc.sync.dma_start(hs[:], hsh[:, j, qsl])
                            nc.vector.scalar_tensor_tensor(
                                h1_t[j][:, qsl], osh[:], bo_t[:, j, :], hs[:],
                                op0=ALU.add, op1=ALU.add)
                            sqc = p78.tile([128, 512], BF16, tag="sqc", bufs=2)
                            nc.scalar.activation(sqc[:], h1_t[j][:, qsl], AF.Square)
                            nc.tensor.matmul(z2[:], ones_red[:], sqc[:],
                                             start=(j == 0), stop=(j == LH - 1))
                        s2row = p78.tile([1, 512], F32, tag="r5", bufs=5)
                        nc.vector.tensor_copy(s2row[:], z2[:])
                        nc.sync.dma_start(s2_in_c[qc][:], s2row[:])
                        nc.gpsimd.collective_compute(
                            "AllReduce", ALU.add, replica_groups=RG,
                            ins=[s2_in_c[qc].opt()], outs=[s2_out_c[qc].opt()])
                        s2f = p78.tile([1, 512], F32, tag="r5", bufs=5)
                        nc.sync.dma_start(s2f[:], s2_out_c[qc][:])
                        rms2 = p78.tile([1, 512], F32, tag="r5", bufs=5)
                        nc.scalar.activation(rms2[:], s2f[:], AF.Sqrt, scale=1.0 / H,
                                             bias=eps_t[:])
                        scl2 = p78.tile([1, 512], F32, tag="r5", bufs=5)
                        nc.vector.reciprocal(scl2[:], rms2[:])
                        sc2b = p78.tile([128, 512], F32, tag="sc2b", bufs=2)
                        nc.gpsimd.partition_broadcast(sc2b[:], scl2[:])
                        for j in range(LH):
                            ysh = p78.tile([128, 512], BF16, tag="ysh", bufs=2)
                            nc.vector.scalar_tensor_tensor(
                                ysh[:], h1_t[j][:, qsl], ln2_t[:, j, :], sc2b[:],
                                op0=ALU.mult, op1=ALU.mult)
                            nc.sync.dma_start(y_in_c[qc][j * 128:(j + 1) * 128, :],
                                              ysh[:])
                        nc.gpsimd.collective_compute(
                            "AllGather", ALU.bypass, replica_groups=RG,
                            ins=[y_in_c[qc].opt()], outs=[y_out_c[qc].opt()])
                        if debug:
                            nc.sync.dma_start(dbg["y_dbg"][:, qsl], y_out_c[qc][:])

            # ================= MLP + chunked RS + out =================
            with tc.tile_pool(name="p9", bufs=1) as p9:
                for ntc in range(NT):
                    tsl = slice(ntc * 512, (ntc + 1) * 512)
                    yk = []
                    for ko in range(KO):
                        t = p9.tile([128, 512], BF16, tag="yk", bufs=KO + 2)
                        nc.sync.dma_start(t[:],
                                          y_out_c[ntc][ko * 128:(ko + 1) * 128, :])
                        yk.append(t)
                    act_sl = p9.tile([128, FFC, 512], BF16, tag="act", bufs=2)
                    for fc in range(FFC):
                        wgt = p9.tile([128, KO, 128], BF16, tag="wgu", bufs=4)
                        nc.sync.dma_start(wgt[:], wg[:, fc, :, :])
                        wut = p9.tile([128, KO, 128], BF16, tag="wgu", bufs=4)
                        nc.sync.dma_start(wut[:], wu[:, fc, :, :])
                        pg = pp.tile([128, 512], F32, tag="pp", bufs=8,
                                     name=f"pg_{ntc}_{fc}")
                        pu = pp.tile([128, 512], F32, tag="pp", bufs=8,
                                     name=f"pu_{ntc}_{fc}")
                        for ko in range(KO):
                            nc.tensor.matmul(pg[:], wgt[:, ko, :], yk[ko][:],
                                             start=(ko == 0), stop=(ko == KO - 1))
                        for ko in range(KO):
                            nc.tensor.matmul(pu[:], wut[:, ko, :], yk[ko][:],
                                             start=(ko == 0), stop=(ko == KO - 1))
                        sg = p9.tile([128, 512], F32, tag="sg", bufs=2)
                        nc.scalar.activation(sg[:], pg[:], AF.Silu)
                        nc.vector.tensor_tensor(act_sl[:, fc, :], pu[:], sg[:],
                                                op=ALU.mult)
                    for mc in range(KO):
                        wdt = p9.tile([128, FFC, 128], BF16, tag="wdt", bufs=4)
                        nc.sync.dma_start(wdt[:], wd[:, mc, :, :])
                        pd = pp.tile([128, 512], F32, tag="pp", bufs=8,
                                     name=f"pd_{ntc}_{mc}")
                        for fc in range(FFC):
                            nc.tensor.matmul(pd[:], wdt[:, fc, :], act_sl[:, fc, :],
                                             start=(fc == 0), stop=(fc == FFC - 1))
                        dd = p9.tile([128, 512], BF16, tag="dd", bufs=4)
                        nc.scalar.copy(dd[:], pd[:])
                        nc.sync.dma_start(d_in_c[ntc][mc * 128:(mc + 1) * 128, :],
                                          dd[:])
                    nc.gpsimd.collective_compute(
                        "ReduceScatter", ALU.add, replica_groups=RG,
                        ins=[d_in_c[ntc].opt()], outs=[d_out_c[ntc].opt()])
                    if debug:
                        nc.sync.dma_start(dbg["mrs_dbg"][:, tsl], d_out_c[ntc][:])
                    for j in range(LH):
                        msh = p9.tile([128, 512], BF16, tag="msh", bufs=3)
                        nc.sync.dma_start(msh[:],
                                          d_out_c[ntc][j * 128:(j + 1) * 128, :])
                        ot = p9.tile([128, 512], F32, tag="outt", bufs=3)
                        nc.vector.tensor_tensor(ot[:], h1_t[j][:, tsl], msh[:],
                                                op=ALU.add)
                        nc.sync.dma_start(out_sh[j * 128:(j + 1) * 128, tsl], ot[:])

    nc.compile()
    return nc


def _feat_major(a):
    """[Hin, M] -> [128, Hin//128, M]"""
    hin, m = a.shape
    return np.ascontiguousarray(a.reshape(hin // 128, 128, m).swapaxes(0, 1))


def _col(b):
    """[512] -> [128, 4, 1]"""
    return np.ascontiguousarray(b.reshape(-1, 128, 1).swapaxes(0, 1))


def _prep_inputs(hidden_states, wq, bq, wk, bk, wv, bv, wo, bo,
                 w_gate, w_up, w_down, ln1_w, ln2_w):
    f32 = np.float32
    hidden = np.asarray(hidden_states, f32)
    hTn = _feat_major(np.ascontiguousarray(hidden.T))           # [128, 32, S]
    ln1 = np.asarray(ln1_w, f32).reshape(KO, 128, 1).swapaxes(0, 1).copy()
    scale = 1.0 / np.sqrt(HD)

    mask = np.zeros((128, 4, 512), f32)
    p = np.arange(128)[:, None, None]
    j = np.arange(4)[None, :, None]
    c = np.arange(512)[None, None, :]
    mask[c >= p + 128 * j] = 1.0
    mask = mask.astype(bfloat16)

    wq_ = np.asarray(wq, f32) * scale
    bq_ = np.asarray(bq, f32) * scale
    wk_, bk_ = np.asarray(wk, f32), np.asarray(bk, f32)
    wv_, bv_ = np.asarray(wv, f32), np.asarray(bv, f32)
    wo_, bo_ = np.asarray(wo, f32), np.asarray(bo, f32)
    wg_, wu_, wdn_ = (np.asarray(w_gate, f32), np.asarray(w_up, f32),
                      np.asarray(w_down, f32))
    ln2 = np.asarray(ln2_w, f32)

    in_maps = []
    for i in range(NC):
        qs = slice(i * QK, (i + 1) * QK)
        fs = slice(i * FFL, (i + 1) * FFL)
        ss = slice(i * SHD, (i + 1) * SHD)
        wo_fm = _feat_major(wo_[:, qs].T).astype(bfloat16)      # [128, 4, 4096]
        wo_r = np.ascontiguousarray(
            wo_fm.reshape(128, LH, KO, 128).transpose(0, 2, 1, 3))
        wg_fm = _feat_major(wg_[fs, :].T).astype(bfloat16)      # [128, 32, 1792]
        wg_r = np.ascontiguousarray(
            wg_fm.reshape(128, KO, FFC, 128).transpose(0, 2, 1, 3))
        wu_fm = _feat_major(wu_[fs, :].T).astype(bfloat16)
        wu_r = np.ascontiguousarray(
            wu_fm.reshape(128, KO, FFC, 128).transpose(0, 2, 1, 3))
        wd_fm = _feat_major(wdn_[:, fs].T).astype(bfloat16)     # [128, 14, 4096]
        wd_r = np.ascontiguousarray(
            wd_fm.reshape(128, FFC, KO, 128).transpose(0, 2, 1, 3))
        m = {
            "hsh": _feat_major(np.ascontiguousarray(hidden.T[ss, :])),
            "hT": hTn,
            "ln1w": ln1,
            "ln2w": _col(ln2[ss]),
            "wq": _feat_major(wq_[qs, :].T).astype(bfloat16),
            "wk": _feat_major(wk_[qs, :].T).astype(bfloat16),
            "wv": _feat_major(wv_[qs, :].T).astype(bfloat16),
            "bq": _col(bq_[qs]),
            "bk": _col(bk_[qs]),
            "bvr": bv_[qs][None, :].astype(bfloat16),
            "wo": wo_r,
            "bo": _col(bo_[ss]),
            "wg": wg_r,
            "wu": wu_r,
            "wd": wd_r,
            "masks": mask,
        }
        in_maps.append(m)
    return in_maps


def run(inputs, debug=False, trace=False):
    key = ("nc", debug)
    if key not in _cache:
        _cache[key] = _build(debug=debug)
    nc = _cache[key]
    in_maps = _prep_inputs(
        inputs["hidden_states"], inputs["wq"], inputs["bq"], inputs["wk"],
        inputs["bk"], inputs["wv"], inputs["bv"], inputs["wo"], inputs["bo"],
        inputs["w_gate"], inputs["w_up"], inputs["w_down"], inputs["ln1_w"],
        inputs["ln2_w"])
    res = run_bass_kernel_spmd(nc, in_maps, core_ids=list(range(NC)), trace=trace)
    shards = [np.asarray(r["out_sh"]) for r in res.results]
    out = np.concatenate(shards, axis=0).T
    return np.ascontiguousarray(out, dtype=np.float32), res


def kernel(**inputs):
    out, _ = run(inputs, debug=False, trace=False)
    return out
